# revision 20
# baseline (speedup 1.0000x reference)
"""YOLOv1 loss kernel for Trainium2, data-parallel over 8 NeuronCores.

Full inputs: pred [16384,30,7,7] f32, labels [16384,30,7,7] f32 -> scalar f32.

Strategy (v3 "compact"):
  Each grid cell (row, i, j) is an independent unit: the loss is a plain sum
  of per-cell terms, and the grid offsets m,n cancel inside the IOU (both
  boxes of a cell shift equally). Cells split into two streams:
    - obj cells (labels[:,4]==1, ~30%): full pipeline (IOU, responsibility,
      coor, conf, cls) over 58 bf16 values/cell, xy-major with label box
      channels duplicated so every op is a contiguous 2D/3D access pattern:
      [x1 x2 y1 y2 | lx lx ly ly | w1 w2 h1 h2 | lw lw lh lh | c1 c2 |
       pcls*20 | lcls*20]
    - noobj cells (~70%): only 0.5*(c1^2 + c2^2), i.e. 2 bf16 values/cell.
  The host packs each stream densely over 128 partitions (bf16 casts,
  channel gather/duplication, padding with exactly-zero-contribution cells);
  all loss arithmetic runs on device. Per-partition f32 partials are summed
  on host (f64) and divided by B.

Engine split: DVE runs the bf16 TT chain (2x packed mode); ScalarE runs
Abs/Sqrt and every Square+accumulate reduction, with the coor weight 5 and
conf weight 0.5 folded into the activation scale ((sqrt(5)x)^2 = 5x^2).
GpSimd is left idle on purpose: its SBUF port is shared with the DVE and
any wide GpSimd op starves the vector engine. Slices are asymmetric so
compute starts as soon as the first (smaller) DMA chunk lands.

Math notes (equivalent to the reference up to bf16 rounding):
  - 1D overlap identity: min(a+p, b+q) - max(a-p, b-q) = (p+q) - max(|a-b|,
    |p-q|); with p=3.5*pw, q=3.5*gw this gives the (x7-scaled) intersection
    side from dxy = px-gx (shared with the coor term).
  - ii = 49*inter, dn = 49*(aa+ag) - ii > 0 always (label w,h >= 0.05), so
    iou = ii/dn needs no zero-guard; 1/dn via reciprocal_approx_fast (f32).
  - On obj cells the conf coefficient of (c_k - iou_k)^2 is resp_k + 0.5*
    (1-resp_k) = 0.5*(1 + resp_k); coor coefficient is 5*resp_k; cls
    unweighted. resp1 = iou1>=iou2, resp2 = iou1<iou2 (exact complement).
"""

import numpy as np
import ml_dtypes

import concourse.mybir as mybir
import concourse.tile as tile
from concourse import bacc
from concourse.bass_utils import run_bass_kernel_spmd

F32 = mybir.dt.float32
BF16 = mybir.dt.bfloat16
OP = mybir.AluOpType
AF = mybir.ActivationFunctionType

NCORES = 8
B = 16384
BLOC = B // NCORES        # 2048 rows per core
S2 = 49
P = 128
NCH = 58                  # obj-stream channels per cell
SFRAC = (3, 5)            # relative slice widths (asymmetric pipeline)

SQ5 = float(np.sqrt(5.0))
ISQ2 = float(np.sqrt(0.5))
SQ2M1 = float(np.sqrt(2.0) - 1.0)

PRED_PERM = [0, 5, 1, 6, 2, 7, 3, 8, 4, 9] + list(range(10, 30))
LAB_PERM = [0, 0, 1, 1, 2, 2, 3, 3] + list(range(10, 30))
# pad cell: identical unit boxes, conf=1 (=iou), zero cls -> contributes 0
PAD_CELL = np.array(
    [0, 0, 0, 0, 0, 0, 0, 0, 1, 1, 1, 1, 1, 1, 1, 1, 1, 1]
    + [0] * 40,
    dtype=np.float32,
)

BF = ml_dtypes.bfloat16


def _slice_widths(ncol):
    tot = sum(SFRAC)
    ws = []
    rem = ncol
    for i, f in enumerate(SFRAC):
        if i == len(SFRAC) - 1:
            w = rem
        else:
            w = max(2, (ncol * f // tot + 1) // 2 * 2)
        ws.append(w)
        rem -= w
    assert all(w > 0 and w % 2 == 0 for w in ws) and sum(ws) == ncol
    return ws


def _body(tc, xo_ap, xn_ap, out_ap, WS, CN):
    nc = tc.nc
    nv = nc.vector
    na = nc.scalar
    NACC = 3 * len(WS) + 1

    import contextlib
    ctx = contextlib.ExitStack()
    with ctx:
        inp = ctx.enter_context(tc.tile_pool(name="inp", bufs=2))

        acc = inp.tile([P, NACC], F32, tag="acc", name="acc")

        # noobj stream: DMA early, Square later (after the first Sqrt so the
        # scalar engine loads its activation table set exactly once)
        XN = inp.tile([P, 2 * CN], BF16, tag="XN", name="XN")
        nc.sync.dma_start(XN[:], xn_ap)
        noobj_pending = [True]

        off = 0
        for s, CW in enumerate(WS):
            X = inp.tile([P, NCH * CW], BF16, tag="X", name="X")
            nc.sync.dma_start(X[:], xo_ap[:, off:off + NCH * CW])
            off += NCH * CW

            def fl(c0, c1):
                # flat [P, (c1-c0)*CW] channel range of X
                return X[:][:, c0 * CW:c1 * CW]

            def t(name, n, dt=BF16):
                tt = inp.tile([P, n * CW], dt, tag=f"{name}{s}",
                              name=f"{name}{s}")
                return tt[:]

            # D: [ds(4) | dxy(4) | e35(4)] supertile:
            #   mall reads [ds|dxy] (8CW), abs reads [dxy|e35] (8CW)
            D = t("D", 12)
            ds = D[:, 0:4 * CW]
            dxy = D[:, 4 * CW:8 * CW]
            e35 = D[:, 8 * CW:12 * CW]

            # ---- diffs + scaled widths ----
            nv.tensor_tensor(dxy, fl(0, 4), fl(4, 8), OP.subtract)
            wh35 = t("wh35", 8)   # 3.5*[w1 w2 h1 h2 | lw lw lh lh]
            nv.tensor_scalar_mul(wh35, fl(8, 16), 3.5)
            p35 = wh35[:, 0:4 * CW]
            l35 = wh35[:, 4 * CW:8 * CW]
            nv.tensor_tensor(e35, p35, l35, OP.subtract)
            s35 = t("s35", 4)
            nv.tensor_tensor(s35, p35, l35, OP.add)

            # ---- overlap: side = relu(s35 - max(|dxy|, |e35|)) ----
            ab = t("ab", 8)
            na.activation(ab, D[:, 4 * CW:12 * CW], AF.Abs)
            m2 = t("m2", 4)
            nv.tensor_tensor(m2, ab[:, 0:4 * CW], ab[:, 4 * CW:8 * CW],
                             OP.max)
            side = t("side", 4)
            nv.tensor_tensor(side, s35, m2, OP.subtract)
            nv.tensor_scalar_max(side, side, 0.0)
            ii = t("ii", 2)       # [i1 i2] = 49*inter
            nv.tensor_tensor(ii, side[:, 0:2 * CW], side[:, 2 * CW:4 * CW],
                             OP.mult)

            # ---- iou = ii / (49*(aa+ag) - ii) ----
            wv = X[:][:, 8 * CW:16 * CW].rearrange(
                "p (g r) -> p g r", g=2)
            aag = t("aag", 4)     # [a1 a2 | ag ag]
            nv.tensor_tensor(aag.rearrange("p (g r) -> p g r", g=2),
                             wv[:, :, 0:2 * CW], wv[:, :, 2 * CW:4 * CW],
                             OP.mult)
            ss = t("ss", 2)
            nv.tensor_tensor(ss, aag[:, 0:2 * CW], aag[:, 2 * CW:4 * CW],
                             OP.add)
            dn = t("dn", 2, dt=F32)
            nv.scalar_tensor_tensor(dn, ss, 49.0, ii, OP.mult, OP.subtract)
            rc = t("rc", 2, dt=F32)
            nv.reciprocal_approx_fast(rc, dn)
            io = t("io", 2)
            nv.tensor_tensor(io, ii, rc, OP.mult)

            # ---- sqrt for coor wh (ds written into D before MW reads) ----
            sq = t("sq", 8)
            na.activation(sq, fl(8, 16), AF.Sqrt)
            nv.tensor_tensor(ds, sq[:, 0:4 * CW], sq[:, 4 * CW:8 * CW],
                             OP.subtract)
            if noobj_pending[0]:
                noobj_pending[0] = False
                scrn = inp.tile([P, 2 * CN], BF16, tag="scrn", name="scrn")
                na.activation(scrn[:], XN[:], AF.Square, scale=ISQ2,
                              accum_out=acc[:, NACC - 1:NACC])

            # ---- responsibility ----
            wt = t("wt", 2)       # [resp1 resp2=1-resp1]
            nv.tensor_tensor(wt[:, 0:CW], io[:, 0:CW], io[:, CW:2 * CW],
                             OP.is_ge)
            nv.tensor_tensor(wt[:, CW:2 * CW], io[:, 0:CW], io[:, CW:2 * CW],
                             OP.is_lt)
            ws = t("ws", 2)       # sqrt(1 + resp); conf 0.5 in Square scale
            nv.tensor_scalar(ws, wt, SQ2M1, 1.0, OP.mult, OP.add)

            # ---- diffs vs targets ----
            t2 = t("t2", 2)
            nv.tensor_tensor(t2, fl(16, 18), io, OP.subtract)
            dc = t("dc", 20)
            nv.tensor_tensor(dc, fl(18, 38), fl(38, 58), OP.subtract)

            # ---- weighted pieces: [tw | mall] ----
            MW = t("MW", 10)      # [tw(2) | (ds dxy)*wt (8)]
            nv.tensor_tensor(MW[:, 0:2 * CW], t2, ws, OP.mult)
            wtb = wt.unsqueeze(1).broadcast_to((P, 4, 2 * CW))
            nv.tensor_tensor(
                MW[:, 2 * CW:10 * CW].rearrange("p (a r) -> p a r", a=4),
                D[:, 0:8 * CW].rearrange("p (a r) -> p a r", a=4),
                wtb, OP.mult)

            # ---- accumulate: conf (x0.5), coor (x5), cls ----
            sc1 = t("sc1", 2)
            na.activation(sc1, MW[:, 0:2 * CW], AF.Square, scale=ISQ2,
                          accum_out=acc[:, 3 * s + 0:3 * s + 1])
            sc2 = t("sc2", 8)
            na.activation(sc2, MW[:, 2 * CW:10 * CW], AF.Square, scale=SQ5,
                          accum_out=acc[:, 3 * s + 1:3 * s + 2])
            sc3 = t("sc3", 20)
            if s == len(WS) - 1:
                nv.scalar_tensor_tensor(
                    sc3, dc, 1.0, dc, OP.mult, OP.mult,
                    accum_out=acc[:, 3 * s + 2:3 * s + 3])
            else:
                na.activation(sc3, dc, AF.Square,
                              accum_out=acc[:, 3 * s + 2:3 * s + 3])

        nc.sync.dma_start(out_ap, acc[:])


_NC_CACHE = {}


def build_nc(WS, CN):
    key = (tuple(WS), CN)
    if key in _NC_CACHE:
        return _NC_CACHE[key]
    nc = bacc.Bacc(
        "TRN2",
        target_bir_lowering=False,
        debug=False,
        enable_asserts=False,
        num_devices=NCORES,
    )
    ncol = sum(WS)
    xo = nc.dram_tensor("xo", [P, NCH * ncol], BF16, kind="ExternalInput")
    xn = nc.dram_tensor("xn", [P, 2 * CN], BF16, kind="ExternalInput")
    out = nc.dram_tensor("out", [P, 3 * len(WS) + 1], F32,
                         kind="ExternalOutput")
    with tile.TileContext(nc) as tc:
        _body(tc, xo.ap(), xn.ap(), out.ap(), WS, CN)
    nc.compile()
    _NC_CACHE[key] = nc
    return nc


def _pack_core(Pc, Lc, m, WS, CN):
    """Pc,Lc: [BLOC*49, 30] f32 per-cell channels; m: bool obj mask."""
    idx1 = np.nonzero(m)[0]
    idx0 = np.nonzero(~m)[0]
    ncol = sum(WS)
    O = np.empty((ncol * P, NCH), dtype=np.float32)
    k1 = len(idx1)
    pp = Pc[idx1][:, PRED_PERM]   # [x1 x2 y1 y2 w1 w2 h1 h2 c1 c2 cls*20]
    ll = Lc[idx1][:, LAB_PERM]    # [lx lx ly ly lw lw lh lh cls*20]
    O[:k1, 0:4] = pp[:, 0:4]      # x1 x2 y1 y2
    O[:k1, 4:8] = ll[:, 0:4]      # lx lx ly ly
    O[:k1, 8:12] = pp[:, 4:8]     # w1 w2 h1 h2
    O[:k1, 12:16] = ll[:, 4:8]    # lw lw lh lh
    O[:k1, 16:18] = pp[:, 8:10]   # c1 c2
    O[:k1, 18:38] = pp[:, 10:30]
    O[:k1, 38:58] = ll[:, 8:28]
    O[k1:] = PAD_CELL
    # cell j -> (col q=j//P, p=j%P); cols split into slices of widths WS
    A = O.reshape(ncol, P, NCH)
    parts = []
    q0 = 0
    for w in WS:
        blk = A[q0:q0 + w].transpose(1, 2, 0)      # [P, NCH, w]
        parts.append(blk.reshape(P, NCH * w))
        q0 += w
    xo = np.ascontiguousarray(np.concatenate(parts, axis=1)).astype(BF)

    k0 = len(idx0)
    N = np.zeros((CN * P, 2), dtype=np.float32)
    N[:k0, 0] = Pc[idx0, 4]
    N[:k0, 1] = Pc[idx0, 9]
    xn = N.reshape(CN, P, 2).transpose(1, 2, 0)
    xn = np.ascontiguousarray(xn).reshape(P, 2 * CN).astype(BF)
    return {"xo": xo, "xn": xn}


def prepare(pred, labels):
    pred = np.asarray(pred, dtype=np.float32).reshape(B, 30, S2)
    labels = np.asarray(labels, dtype=np.float32).reshape(B, 30, S2)
    masks = []
    Pcs = []
    Lcs = []
    k1s = []
    for c in range(NCORES):
        r0 = c * BLOC
        Pc = np.ascontiguousarray(
            pred[r0:r0 + BLOC].transpose(0, 2, 1)).reshape(-1, 30)
        Lc = np.ascontiguousarray(
            labels[r0:r0 + BLOC].transpose(0, 2, 1)).reshape(-1, 30)
        m = Lc[:, 4] == 1.0
        masks.append(m)
        Pcs.append(Pc)
        Lcs.append(Lc)
        k1s.append(int(m.sum()))
    k1max = max(k1s)
    k0max = max(BLOC * S2 - k1 for k1 in k1s)

    def cdiv(a, b):
        return -(-a // b)

    ncol = max(4, cdiv(cdiv(k1max, P), 2) * 2)
    WS = _slice_widths(ncol)
    CN = max(2, cdiv(cdiv(k0max, P), 2) * 2)
    nc = build_nc(WS, CN)
    in_maps = [
        _pack_core(Pcs[c], Lcs[c], masks[c], WS, CN) for c in range(NCORES)
    ]
    return nc, in_maps


def run(pred, labels, trace=False, **kw):
    nc, in_maps = prepare(pred, labels)
    res = run_bass_kernel_spmd(
        nc, in_maps, core_ids=list(range(NCORES)), trace=trace, **kw)
    total = np.float64(0.0)
    for r in res.results:
        total += r["out"].astype(np.float64).sum()
    loss = np.float32(total / B)
    return loss, res


def kernel(pred, labels):
    loss, _ = run(pred, labels)
    return np.array(loss, dtype=np.float32)


# revision 21
# speedup vs baseline: 1.1239x; 1.1239x over previous
"""YOLOv1 loss kernel for Trainium2, data-parallel over 8 NeuronCores.

Full inputs: pred [16384,30,7,7] f32, labels [16384,30,7,7] f32 -> scalar f32.

Strategy (v3 "compact"):
  Each grid cell (row, i, j) is an independent unit: the loss is a plain sum
  of per-cell terms, and the grid offsets m,n cancel inside the IOU (both
  boxes of a cell shift equally). Cells split into two streams:
    - obj cells (labels[:,4]==1, ~30%): full pipeline (IOU, responsibility,
      coor, conf, cls) over 58 bf16 values/cell, xy-major with label box
      channels duplicated so every op is a contiguous 2D/3D access pattern:
      [x1 x2 y1 y2 | lx lx ly ly | w1 w2 h1 h2 | lw lw lh lh | c1 c2 |
       pcls*20 | lcls*20]
    - noobj cells (~70%): only 0.5*(c1^2 + c2^2), i.e. 2 bf16 values/cell.
  The host packs each stream densely over 128 partitions (bf16 casts,
  channel gather/duplication, padding with exactly-zero-contribution cells);
  all loss arithmetic runs on device. Per-partition f32 partials are summed
  on host (f64) and divided by B.

Engine split: DVE runs the bf16 TT chain (2x packed mode); ScalarE runs
Abs/Sqrt and every Square+accumulate reduction, with the coor weight 5 and
conf weight 0.5 folded into the activation scale ((sqrt(5)x)^2 = 5x^2).
GpSimd is left idle on purpose: its SBUF port is shared with the DVE and
any wide GpSimd op starves the vector engine. Slices are asymmetric so
compute starts as soon as the first (smaller) DMA chunk lands.

Math notes (equivalent to the reference up to bf16 rounding):
  - 1D overlap identity: min(a+p, b+q) - max(a-p, b-q) = (p+q) - max(|a-b|,
    |p-q|); with p=3.5*pw, q=3.5*gw this gives the (x7-scaled) intersection
    side from dxy = px-gx (shared with the coor term).
  - ii = 49*inter, dn = 49*(aa+ag) - ii > 0 always (label w,h >= 0.05), so
    iou = ii/dn needs no zero-guard; 1/dn via reciprocal_approx_fast (f32).
  - On obj cells the conf coefficient of (c_k - iou_k)^2 is resp_k + 0.5*
    (1-resp_k) = 0.5*(1 + resp_k); coor coefficient is 5*resp_k; cls
    unweighted. resp1 = iou1>=iou2, resp2 = iou1<iou2 (exact complement).
"""

import numpy as np
import ml_dtypes

import concourse.mybir as mybir
import concourse.tile as tile
from concourse import bacc
from concourse.bass_utils import run_bass_kernel_spmd

F32 = mybir.dt.float32
BF16 = mybir.dt.bfloat16
OP = mybir.AluOpType
AF = mybir.ActivationFunctionType

NCORES = 8
B = 16384
BLOC = B // NCORES        # 2048 rows per core
S2 = 49
P = 128
NCH = 58                  # obj-stream channels per cell
SFRAC = (3, 5)            # relative slice widths (asymmetric pipeline)

SQ5 = float(np.sqrt(5.0))
ISQ2 = float(np.sqrt(0.5))
SQ2M1 = float(np.sqrt(2.0) - 1.0)

PRED_PERM = [0, 5, 1, 6, 2, 7, 3, 8, 4, 9] + list(range(10, 30))
LAB_PERM = [0, 0, 1, 1, 2, 2, 3, 3] + list(range(10, 30))
# pad cell: identical unit boxes, conf=1 (=iou), zero cls -> contributes 0
PAD_CELL = np.array(
    [0, 0, 0, 0, 0, 0, 0, 0, 1, 1, 1, 1, 1, 1, 1, 1, 1, 1]
    + [0] * 40,
    dtype=np.float32,
)

BF = ml_dtypes.bfloat16


def _slice_widths(ncol):
    tot = sum(SFRAC)
    ws = []
    rem = ncol
    for i, f in enumerate(SFRAC):
        if i == len(SFRAC) - 1:
            w = rem
        else:
            w = max(2, (ncol * f // tot + 1) // 2 * 2)
        ws.append(w)
        rem -= w
    assert all(w > 0 and w % 2 == 0 for w in ws) and sum(ws) == ncol
    return ws


def _body(tc, xo_ap, xn_ap, out_ap, WS, CN):
    nc = tc.nc
    nv = nc.vector
    na = nc.scalar
    NACC = 3 * len(WS) + 1

    import contextlib
    ctx = contextlib.ExitStack()
    with ctx:
        inp = ctx.enter_context(tc.tile_pool(name="inp", bufs=2))
        opool = ctx.enter_context(tc.tile_pool(name="opool", bufs=1))

        acc = opool.tile([P, NACC], F32, tag="acc", name="acc")

        # ---- noobj stream first: sum 0.5*(c1^2 + c2^2) ----
        XN = inp.tile([P, 2 * CN], BF16, tag="XN", name="XN")
        nc.sync.dma_start(XN[:], xn_ap)
        scrn = inp.tile([P, 2 * CN], BF16, tag="scrn", name="scrn")
        na.activation(scrn[:], XN[:], AF.Square, scale=ISQ2,
                      accum_out=acc[:, NACC - 1:NACC])

        off = 0
        for s, CW in enumerate(WS):
            X = inp.tile([P, NCH * CW], BF16, tag="X", name="X")
            nc.sync.dma_start(X[:], xo_ap[:, off:off + NCH * CW])
            off += NCH * CW

            def fl(c0, c1):
                # flat [P, (c1-c0)*CW] channel range of X
                return X[:][:, c0 * CW:c1 * CW]

            def t(name, n, dt=BF16):
                tt = inp.tile([P, n * CW], dt, tag=f"{name}{s}",
                              name=f"{name}{s}")
                return tt[:]

            # D: [ds(4) | dxy(4) | e35(4)] supertile:
            #   mall reads [ds|dxy] (8CW), abs reads [dxy|e35] (8CW)
            D = t("D", 12)
            ds = D[:, 0:4 * CW]
            dxy = D[:, 4 * CW:8 * CW]
            e35 = D[:, 8 * CW:12 * CW]

            # ---- diffs + scaled widths ----
            nv.tensor_tensor(dxy, fl(0, 4), fl(4, 8), OP.subtract)
            wh35 = t("wh35", 8)   # 3.5*[w1 w2 h1 h2 | lw lw lh lh]
            nv.tensor_scalar_mul(wh35, fl(8, 16), 3.5)
            p35 = wh35[:, 0:4 * CW]
            l35 = wh35[:, 4 * CW:8 * CW]
            nv.tensor_tensor(e35, p35, l35, OP.subtract)
            s35 = t("s35", 4)
            nv.tensor_tensor(s35, p35, l35, OP.add)

            # ---- overlap: side = relu(s35 - max(|dxy|, |e35|)) ----
            ab = t("ab", 8)
            na.activation(ab, D[:, 4 * CW:12 * CW], AF.Abs)
            m2 = t("m2", 4)
            nv.tensor_tensor(m2, ab[:, 0:4 * CW], ab[:, 4 * CW:8 * CW],
                             OP.max)
            side = t("side", 4)
            nv.tensor_tensor(side, s35, m2, OP.subtract)
            nv.tensor_scalar_max(side, side, 0.0)
            ii = t("ii", 2)       # [i1 i2] = 49*inter
            nv.tensor_tensor(ii, side[:, 0:2 * CW], side[:, 2 * CW:4 * CW],
                             OP.mult)

            # ---- iou = ii / (49*(aa+ag) - ii) ----
            wv = X[:][:, 8 * CW:16 * CW].rearrange(
                "p (g r) -> p g r", g=2)
            aag = t("aag", 4)     # [a1 a2 | ag ag]
            nv.tensor_tensor(aag.rearrange("p (g r) -> p g r", g=2),
                             wv[:, :, 0:2 * CW], wv[:, :, 2 * CW:4 * CW],
                             OP.mult)
            ss = t("ss", 2)
            nv.tensor_tensor(ss, aag[:, 0:2 * CW], aag[:, 2 * CW:4 * CW],
                             OP.add)
            dn = t("dn", 2, dt=F32)
            nv.scalar_tensor_tensor(dn, ss, 49.0, ii, OP.mult, OP.subtract)
            rc = t("rc", 2, dt=F32)
            nv.reciprocal_approx_fast(rc, dn)
            io = t("io", 2)
            nv.tensor_tensor(io, ii, rc, OP.mult)

            # ---- sqrt for coor wh (ds written into D before MW reads) ----
            sq = t("sq", 8)
            na.activation(sq, fl(8, 16), AF.Sqrt)
            nv.tensor_tensor(ds, sq[:, 0:4 * CW], sq[:, 4 * CW:8 * CW],
                             OP.subtract)

            # ---- responsibility ----
            wt = t("wt", 2)       # [resp1 resp2=1-resp1]
            nv.tensor_tensor(wt[:, 0:CW], io[:, 0:CW], io[:, CW:2 * CW],
                             OP.is_ge)
            nv.tensor_tensor(wt[:, CW:2 * CW], io[:, 0:CW], io[:, CW:2 * CW],
                             OP.is_lt)
            ws = t("ws", 2)       # sqrt(1 + resp); conf 0.5 in Square scale
            nv.tensor_scalar(ws, wt, SQ2M1, 1.0, OP.mult, OP.add)

            # ---- diffs vs targets ----
            t2 = t("t2", 2)
            nv.tensor_tensor(t2, fl(16, 18), io, OP.subtract)
            dc = t("dc", 20)
            nv.tensor_tensor(dc, fl(18, 38), fl(38, 58), OP.subtract)

            # ---- weighted pieces: [tw | mall] ----
            MW = t("MW", 10)      # [tw(2) | (ds dxy)*wt (8)]
            nv.tensor_tensor(MW[:, 0:2 * CW], t2, ws, OP.mult)
            wtb = wt.unsqueeze(1).broadcast_to((P, 4, 2 * CW))
            nv.tensor_tensor(
                MW[:, 2 * CW:10 * CW].rearrange("p (a r) -> p a r", a=4),
                D[:, 0:8 * CW].rearrange("p (a r) -> p a r", a=4),
                wtb, OP.mult)

            # ---- accumulate: conf (x0.5), coor (x5), cls ----
            sc1 = t("sc1", 2)
            na.activation(sc1, MW[:, 0:2 * CW], AF.Square, scale=ISQ2,
                          accum_out=acc[:, 3 * s + 0:3 * s + 1])
            sc2 = t("sc2", 8)
            na.activation(sc2, MW[:, 2 * CW:10 * CW], AF.Square, scale=SQ5,
                          accum_out=acc[:, 3 * s + 1:3 * s + 2])
            sc3 = t("sc3", 20)
            na.activation(sc3, dc, AF.Square,
                          accum_out=acc[:, 3 * s + 2:3 * s + 3])

        nc.sync.dma_start(out_ap, acc[:])


_NC_CACHE = {}


def build_nc(WS, CN):
    key = (tuple(WS), CN)
    if key in _NC_CACHE:
        return _NC_CACHE[key]
    nc = bacc.Bacc(
        "TRN2",
        target_bir_lowering=False,
        debug=False,
        enable_asserts=False,
        num_devices=NCORES,
    )
    ncol = sum(WS)
    xo = nc.dram_tensor("xo", [P, NCH * ncol], BF16, kind="ExternalInput")
    xn = nc.dram_tensor("xn", [P, 2 * CN], BF16, kind="ExternalInput")
    out = nc.dram_tensor("out", [P, 3 * len(WS) + 1], F32,
                         kind="ExternalOutput")
    with tile.TileContext(nc) as tc:
        _body(tc, xo.ap(), xn.ap(), out.ap(), WS, CN)
    nc.compile()
    _NC_CACHE[key] = nc
    return nc


def _pack_core(Pc, Lc, m, WS, CN):
    """Pc,Lc: [BLOC*49, 30] f32 per-cell channels; m: bool obj mask."""
    idx1 = np.nonzero(m)[0]
    idx0 = np.nonzero(~m)[0]
    ncol = sum(WS)
    O = np.empty((ncol * P, NCH), dtype=np.float32)
    k1 = len(idx1)
    pp = Pc[idx1][:, PRED_PERM]   # [x1 x2 y1 y2 w1 w2 h1 h2 c1 c2 cls*20]
    ll = Lc[idx1][:, LAB_PERM]    # [lx lx ly ly lw lw lh lh cls*20]
    O[:k1, 0:4] = pp[:, 0:4]      # x1 x2 y1 y2
    O[:k1, 4:8] = ll[:, 0:4]      # lx lx ly ly
    O[:k1, 8:12] = pp[:, 4:8]     # w1 w2 h1 h2
    O[:k1, 12:16] = ll[:, 4:8]    # lw lw lh lh
    O[:k1, 16:18] = pp[:, 8:10]   # c1 c2
    O[:k1, 18:38] = pp[:, 10:30]
    O[:k1, 38:58] = ll[:, 8:28]
    O[k1:] = PAD_CELL
    # cell j -> (col q=j//P, p=j%P); cols split into slices of widths WS
    A = O.reshape(ncol, P, NCH)
    parts = []
    q0 = 0
    for w in WS:
        blk = A[q0:q0 + w].transpose(1, 2, 0)      # [P, NCH, w]
        parts.append(blk.reshape(P, NCH * w))
        q0 += w
    xo = np.ascontiguousarray(np.concatenate(parts, axis=1)).astype(BF)

    k0 = len(idx0)
    N = np.zeros((CN * P, 2), dtype=np.float32)
    N[:k0, 0] = Pc[idx0, 4]
    N[:k0, 1] = Pc[idx0, 9]
    xn = N.reshape(CN, P, 2).transpose(1, 2, 0)
    xn = np.ascontiguousarray(xn).reshape(P, 2 * CN).astype(BF)
    return {"xo": xo, "xn": xn}


def prepare(pred, labels):
    pred = np.asarray(pred, dtype=np.float32).reshape(B, 30, S2)
    labels = np.asarray(labels, dtype=np.float32).reshape(B, 30, S2)
    masks = []
    Pcs = []
    Lcs = []
    k1s = []
    for c in range(NCORES):
        r0 = c * BLOC
        Pc = np.ascontiguousarray(
            pred[r0:r0 + BLOC].transpose(0, 2, 1)).reshape(-1, 30)
        Lc = np.ascontiguousarray(
            labels[r0:r0 + BLOC].transpose(0, 2, 1)).reshape(-1, 30)
        m = Lc[:, 4] == 1.0
        masks.append(m)
        Pcs.append(Pc)
        Lcs.append(Lc)
        k1s.append(int(m.sum()))
    k1max = max(k1s)
    k0max = max(BLOC * S2 - k1 for k1 in k1s)

    def cdiv(a, b):
        return -(-a // b)

    ncol = max(4, cdiv(cdiv(k1max, P), 2) * 2)
    WS = _slice_widths(ncol)
    CN = max(2, cdiv(cdiv(k0max, P), 2) * 2)
    nc = build_nc(WS, CN)
    in_maps = [
        _pack_core(Pcs[c], Lcs[c], masks[c], WS, CN) for c in range(NCORES)
    ]
    return nc, in_maps


def run(pred, labels, trace=False, **kw):
    nc, in_maps = prepare(pred, labels)
    res = run_bass_kernel_spmd(
        nc, in_maps, core_ids=list(range(NCORES)), trace=trace, **kw)
    total = np.float64(0.0)
    for r in res.results:
        total += r["out"].astype(np.float64).sum()
    loss = np.float32(total / B)
    return loss, res


def kernel(pred, labels):
    loss, _ = run(pred, labels)
    return np.array(loss, dtype=np.float32)
